# revision 1
# baseline (speedup 1.0000x reference)
# Deformable-attention Trainium2 kernel (8-core SPMD, data-parallel over B*2 half-batches).
#
# Per core: half a batch (2048 query pixels). Keys/values (2048 per half) are
# computed locally and exchanged with the pair core via AllGather, so each core
# attends its 2048 queries over the full 4096 keys of its batch.
#
# Layouts:
#   channel-major tensors: [C(part), pixels(free)]
#   key slot order per half: j = l*16 + tile  <->  pixel p = tile*128 + l
#   (slot order is a fixed permutation of pixels; softmax/AV are invariant)
import numpy as np
import ml_dtypes
import concourse.bass as bass
import concourse.tile as tile
from concourse import bacc, mybir
from concourse.bass_utils import run_bass_kernel_spmd

F32 = mybir.dt.float32
F32R = mybir.dt.float32r
BF16 = mybir.dt.bfloat16
I16 = mybir.dt.int16
AF = mybir.ActivationFunctionType
OP = mybir.AluOpType

B, C, H, W = 4, 64, 64, 64
HP = 2048          # pixels per half
NROWS = 34         # q rows incl 1-row halo each side
QCOLS = NROWS * 64 # 2176
PADC = NROWS * 66  # q_pad cols (66-wide rows)
MAGIC = 12582912.0 # 1.5*2^23 round-to-nearest trick
EPS = 1e-5


def sl2(ap, k):
    # view [P, 2*t] as [P, t] selecting coord k (step-2 columns)
    return ap.rearrange("p (t c) -> p t c", c=2)[:, :, k]


def build_program(debug=False):
    nc = bacc.Bacc("TRN2", target_bir_lowering=False, debug=False)

    IN = {}
    def din(name, shape, dt):
        IN[name] = nc.dram_tensor(name, list(shape), dt, kind="ExternalInput")
        return IN[name]

    # per-core data
    din("prompt_rows", (64, QCOLS), BF16)
    din("bq_map", (64, QCOLS), F32)
    din("kv2", (128, 4096), F32)
    din("refmap", (128, 32), F32)
    # shared weights/constants
    din("wqT", (64, 64), BF16)
    din("dw_diag", (64, 9 * 64), BF16)
    din("off_rhs", (64, 2), BF16)
    din("ones_top", (128, 1), BF16)
    din("ones_bot", (128, 1), BF16)
    din("b1", (1, 64), BF16)
    din("bneg", (1, 64), BF16)
    din("selAB", (4, 128), BF16)
    din("selCD", (4, 128), BF16)
    din("wkT2", (128, 64), BF16)
    din("wvT2", (128, 64), BF16)
    din("woT", (64, 64), BF16)
    din("ident", (128, 128), F32)
    din("dwb_vec", (64, 1), F32)
    din("lnw_vec", (64, 1), F32)
    din("lnb_vec", (64, 1), F32)
    din("bo2_vec", (64, 1), F32)
    din("eps_vec", (1, 1), F32)

    out_half = nc.dram_tensor("out_half", [64, HP], F32, kind="ExternalOutput")
    DBG = {}
    def dbg(name, shape, dt=F32):
        if not debug:
            return None
        DBG[name] = nc.dram_tensor(name, list(shape), dt, kind="ExternalOutput")
        return DBG[name]

    def dump(name, t_ap):
        if debug and name in DBG:
            if t_ap.dtype == F32:
                nc.sync.dma_start(DBG[name].ap(), t_ap)
            else:
                nc.gpsimd.dma_start(DBG[name].ap(), t_ap)

    dbg("d_qhalo", (64, QCOLS))
    dbg("d_tt2", (128, HP))
    dbg("d_tgelu", (64, HP))
    dbg("d_tcoord", (128, 32))
    dbg("d_wwide", (128, 64))
    dbg("d_idxwide", (128, 64))
    dbg("d_xab", (128, HP))
    dbg("d_x2sum", (128, HP))
    dbg("d_kstack", (64, 4096))
    dbg("d_vt", (128, 2 * 16 * 65))
    dbg("d_rowsum", (1, 1024))
    dbg("d_s0", (128, 1024))
    dbg("d_e0", (128, 1024))

    with tile.TileContext(nc) as tc:
        with (
            tc.tile_pool(name="cst", bufs=1) as cst,
            tc.tile_pool(name="big", bufs=1) as big,
            tc.tile_pool(name="dram", bufs=1, space="DRAM") as dram,
        ):
            # ---- load constants/weights ----
            ct = {}
            for nm in ["wqT", "dw_diag", "off_rhs", "ones_top", "ones_bot", "b1", "bneg", "selAB",
                       "selCD", "wkT2", "wvT2", "woT", "ident", "dwb_vec", "lnw_vec",
                       "lnb_vec", "bo2_vec", "eps_vec", "refmap", "bq_map"]:
                ct[nm] = cst.tile(list(IN[nm].shape), IN[nm].dtype, tag=nm, name="c_" + nm)
                nc.sync.dma_start(ct[nm][:], IN[nm].ap())

            q2 = big.tile([64, HP], BF16, name="q2")
            kstack = big.tile([64, 4096], BF16, name="kstack")
            vt_all = big.tile([128, 2 * 16 * 65], BF16, name="vt_all")

            with nc.named_scope("keys"):
                with (
                    tc.tile_pool(name="kp", bufs=2, space="PSUM") as kp,
                    tc.tile_pool(name="kp2", bufs=1, space="PSUM") as kp2,
                    tc.tile_pool(name="ksb", bufs=1) as ksb,
                ):
                    prompt_sb = ksb.tile([64, QCOLS], BF16, tag="prompt_sb", name="prompt_sb")
                    nc.sync.dma_start(prompt_sb[:], IN["prompt_rows"].ap())
                    kv2 = ksb.tile([128, 4096], F32, tag="kv2", name="kv2")
                    nc.scalar.dma_start(kv2[0:64, :], IN["kv2"].ap()[0:64, :])
                    nc.scalar.dma_start(kv2[64:128, :], IN["kv2"].ap()[64:128, :])
                    q_halo = ksb.tile([64, QCOLS], BF16, tag="q_halo", name="q_halo")
                    q_pad = ksb.tile([64, PADC], BF16, tag="q_pad", name="q_pad")
                    tt2 = ksb.tile([128, HP], BF16, tag="tt2", name="tt2")
                    t_gelu = ksb.tile([64, HP], BF16, tag="t_gelu", name="t_gelu")
                    x2sum = ksb.tile([128, HP], BF16, tag="x2sum", name="x2sum")
                    # ---- P1: q = wq @ prompt + bq (masked halo bias) ----
                    nc.vector.memset(q_pad[:], 0.0)
                    for c0 in range(0, QCOLS, 512):
                        w_ = min(512, QCOLS - c0)
                        pq = kp.tile([64, 512], F32, tag="pa", name="pq")
                        nc.tensor.matmul(pq[:, :w_], ct["wqT"][:], prompt_sb[:, c0:c0 + w_],
                                         start=True, stop=True)
                        nc.vector.tensor_tensor(q_halo[:, c0:c0 + w_], pq[:, :w_],
                                                ct["bq_map"][:, c0:c0 + w_], OP.add)
                    # interior copy into padded layout (rows at stride 66, col offset 1)
                    qsrc = q_halo[:].rearrange("p (r w) -> p r w", w=64)
                    qdst = q_pad[:].rearrange("p (r w) -> p r w", w=66)[:, :, 1:65]
                    nc.vector.tensor_copy(qdst, qsrc)
                    dump("d_qhalo", q_halo[:])
                    # q duplicated on 128 partitions for row-packed QK
                    nc.vector.tensor_copy(q2[0:64, :], q_halo[:, 64:64 + HP])

                    # ---- P2: depthwise 3x3 as 9 diag matmuls ----
                    qpad3 = q_pad[:].rearrange("p (r w) -> p r w", w=66)
                    for c in range(4):  # output row blocks of 8 rows = 512 px
                        pt_ = kp.tile([64, 512], F32, tag="pa", name="pt")
                        for tap in range(9):
                            dy, dx = tap // 3, tap % 3
                            rhs = qpad3[:, 8 * c + dy: 8 * c + dy + 8, dx:dx + 64]
                            nc.tensor.matmul(pt_[:], ct["dw_diag"][:, 64 * tap:64 * (tap + 1)],
                                             rhs, start=(tap == 0), stop=(tap == 8))
                        sl = slice(512 * c, 512 * (c + 1))
                        nc.scalar.activation(tt2[0:64, sl], pt_[:], AF.Identity, bias=ct["dwb_vec"][:])
                        nc.scalar.activation(tt2[64:128, sl], pt_[:], AF.Square, bias=ct["dwb_vec"][:])
                    dump("d_tt2", tt2[:])

                    # ---- P3: LN stats ----
                    s_sum = ksb.tile([1, HP], F32, tag="s_sum", name="s_sum")
                    s_sq = ksb.tile([1, HP], F32, tag="rowtmp3", name="s_sq")
                    for c0 in range(0, HP, 512):
                        ps_sc = kp.tile([1, 512], F32, tag="pa", name="ps_sc")
                        nc.tensor.matmul(ps_sc[:], ct["ones_top"][:],
                                         tt2[:, c0:c0 + 512], start=True, stop=True)
                        nc.vector.tensor_copy(s_sum[:, c0:c0 + 512], ps_sc[:])
                        ps_sc2 = kp.tile([1, 512], F32, tag="pb", name="ps_sc2")
                        nc.tensor.matmul(ps_sc2[:], ct["ones_bot"][:],
                                         tt2[:, c0:c0 + 512], start=True, stop=True)
                        nc.vector.tensor_copy(s_sq[:, c0:c0 + 512], ps_sc2[:])
                    r_a = ksb.tile([1, HP], F32, tag="rowtmp", name="r_a")
                    nc.scalar.activation(r_a[:], s_sum[:], AF.Square, scale=0.125)
                    r_d = ksb.tile([1, HP], F32, tag="rowtmp2", name="r_d")
                    nc.vector.tensor_tensor(r_d[:], s_sq[:], r_a[:], OP.subtract)
                    r_sq = ksb.tile([1, HP], F32, tag="rowtmp", name="r_sq")
                    nc.scalar.activation(r_sq[:], r_d[:], AF.Sqrt, scale=1.0 / 64.0, bias=ct["eps_vec"][:])
                    r_scr = ksb.tile([1, HP], F32, tag="rowtmp2", name="r_scr")
                    r_stdf = ksb.tile([1, HP], F32, tag="rowtmp4", name="r_stdf")
                    nc.vector.reciprocal_approx_accurate(r_stdf[:], r_sq[:], r_scr[:])
                    r_std = ksb.tile([1, HP], BF16, tag="r_std", name="r_std")
                    nc.vector.tensor_copy(r_std[:], r_stdf[:])
                    r_p = ksb.tile([1, HP], BF16, tag="r_p", name="r_p")
                    nc.vector.tensor_tensor(r_p[:], s_sum[:], r_stdf[:], OP.mult)

                    # ---- P4: normalize + gelu ----
                    for c0 in range(0, HP, 512):
                        sl = slice(c0, c0 + 512)
                        pr = kp.tile([64, 512], F32, tag="pa", name="pr")
                        nc.tensor.matmul(pr[:], ct["b1"][:], r_std[:, sl], start=True, stop=True)
                        pm = kp.tile([64, 512], F32, tag="pb", name="pm")
                        nc.tensor.matmul(pm[:], ct["bneg"][:], r_p[:, sl], start=True, stop=True)
                        x1 = ksb.tile([64, 512], F32, tag="x1", name="x1")
                        nc.vector.tensor_tensor(x1[:], tt2[0:64, sl], pr[:], OP.mult)
                        x2_ = ksb.tile([64, 512], F32, tag="x2_", name="x2_")
                        nc.vector.tensor_tensor(x2_[:], x1[:], pm[:], OP.add)
                        nc.scalar.activation(t_gelu[:, sl], x2_[:], AF.Gelu,
                                             scale=ct["lnw_vec"][:], bias=ct["lnb_vec"][:])
                    dump("d_tgelu", t_gelu[:])

                    # ---- P5: offsets (transposed tiles) + tanh ----
                    ps_off = kp2.tile([128, 32], F32, tag="ps_off", name="ps_off")
                    for i in range(16):
                        nc.tensor.matmul(ps_off[:, 2 * i:2 * i + 2],
                                         t_gelu[:, 128 * i:128 * (i + 1)], ct["off_rhs"][:],
                                         start=True, stop=True)
                    tcoord = ksb.tile([128, 32], F32, tag="tcoord", name="tcoord")
                    nc.scalar.activation(tcoord[:], ps_off[:], AF.Tanh)
                    dump("d_tcoord", tcoord[:])

                    # ---- P6: coords/weights/indices (wide layout) ----
                    def wt(tag):
                        return ksb.tile([128, 32], F32, tag="w" + tag, name="w" + tag)
                    g = wt("g"); nc.vector.scalar_tensor_tensor(g[:], tcoord[:], 2.0, ct["refmap"][:], OP.mult, OP.add)
                    f_ = wt("f"); nc.vector.tensor_scalar(f_[:], g[:], -0.5, MAGIC, OP.add, OP.add)
                    nc.vector.tensor_scalar(f_[:], f_[:], MAGIC, None, OP.subtract)
                    fr = wt("fr"); nc.vector.tensor_tensor(fr[:], g[:], f_[:], OP.subtract)
                    i0 = wt("i0"); nc.vector.tensor_scalar(i0[:], f_[:], 0.0, 63.0, OP.max, OP.min)
                    i1 = wt("i1"); nc.vector.tensor_scalar(i1[:], f_[:], 1.0, 0.0, OP.add, OP.max)
                    nc.vector.tensor_scalar(i1[:], i1[:], 63.0, None, OP.min)
                    mA = wt("mA"); nc.vector.tensor_tensor(mA[:], i0[:], f_[:], OP.is_equal)
                    mB = wt("mB"); nc.vector.tensor_tensor(mB[:], i1[:], f_[:], OP.subtract)
                    nc.vector.tensor_scalar(mB[:], mB[:], 1.0, None, OP.is_equal)
                    om = wt("om"); nc.vector.tensor_scalar(om[:], fr[:], -1.0, 1.0, OP.mult, OP.add)

                    u16 = lambda tag: ksb.tile([128, 16], F32, tag="u" + tag, name="u" + tag)
                    uy0 = u16("y0"); nc.vector.tensor_tensor(uy0[:], sl2(om[:], 0), sl2(mA[:], 0), OP.mult)
                    uy1 = u16("y1"); nc.vector.tensor_tensor(uy1[:], sl2(fr[:], 0), sl2(mB[:], 0), OP.mult)
                    ux0 = u16("x0"); nc.vector.tensor_tensor(ux0[:], sl2(om[:], 1), sl2(mA[:], 1), OP.mult)
                    ux1 = u16("x1"); nc.vector.tensor_tensor(ux1[:], sl2(fr[:], 1), sl2(mB[:], 1), OP.mult)

                    wwide = ksb.tile([128, 64], F32, tag="wwide", name="wwide")
                    nc.vector.tensor_tensor(wwide[:, 0:16], uy0[:], ux0[:], OP.mult)
                    nc.vector.tensor_tensor(wwide[:, 16:32], uy0[:], ux1[:], OP.mult)
                    nc.vector.tensor_tensor(wwide[:, 32:48], uy1[:], ux0[:], OP.mult)
                    nc.vector.tensor_tensor(wwide[:, 48:64], uy1[:], ux1[:], OP.mult)
                    dump("d_wwide", wwide[:])

                    iy0 = u16("iy0"); nc.vector.tensor_scalar(iy0[:], sl2(i0[:], 0), 64.0, None, OP.mult)
                    iy1 = u16("iy1"); nc.vector.tensor_scalar(iy1[:], sl2(i1[:], 0), 64.0, None, OP.mult)
                    idxwide = ksb.tile([128, 64], F32, tag="idxwide", name="idxwide")
                    nc.vector.tensor_tensor(idxwide[:, 0:16], iy0[:], sl2(i0[:], 1), OP.add)
                    nc.vector.tensor_tensor(idxwide[:, 16:32], iy0[:], sl2(i1[:], 1), OP.add)
                    nc.vector.tensor_tensor(idxwide[:, 32:48], iy1[:], sl2(i0[:], 1), OP.add)
                    nc.vector.tensor_tensor(idxwide[:, 48:64], iy1[:], sl2(i1[:], 1), OP.add)
                    dump("d_idxwide", idxwide[:])

                    # ---- P7: transpose to wrapped layouts ----
                    # replicate along free dim first: [A A A A B B B B], [C C C C D D D D]
                    rep1 = ksb.tile([128, 128], F32, tag="rep1", name="rep1")
                    rep2 = ksb.tile([128, 128], F32, tag="rep2", name="rep2")
                    for gix in range(4):
                        nc.vector.tensor_copy(rep1[:, 16 * gix:16 * gix + 16], idxwide[:, 0:16])
                        nc.vector.tensor_copy(rep1[:, 64 + 16 * gix:80 + 16 * gix], idxwide[:, 16:32])
                        nc.vector.tensor_copy(rep2[:, 16 * gix:16 * gix + 16], idxwide[:, 32:48])
                        nc.vector.tensor_copy(rep2[:, 64 + 16 * gix:80 + 16 * gix], idxwide[:, 48:64])
                    ps_iT = kp2.tile([128, 128], F32, tag="ps_iT", name="ps_iT")
                    nc.tensor.transpose(ps_iT[:], rep1[:], ct["ident"][:])
                    idx1 = ksb.tile([128, 128], I16, tag="idx1", name="idx1")
                    nc.vector.tensor_copy(idx1[:], ps_iT[:])
                    ps_iT2 = kp2.tile([128, 128], F32, tag="ps_iT", name="ps_iT2")
                    nc.tensor.transpose(ps_iT2[:], rep2[:], ct["ident"][:])
                    idx2 = ksb.tile([128, 128], I16, tag="idx2", name="idx2")
                    nc.vector.tensor_copy(idx2[:], ps_iT2[:])

                    ps_wT = kp2.tile([64, 128], F32, tag="ps_iT", name="ps_wT")
                    nc.tensor.transpose(ps_wT[:], wwide[:], ct["ident"][:])
                    wT4 = ksb.tile([64, 128], BF16, tag="wT4", name="wT4")
                    nc.vector.tensor_copy(wT4[:], ps_wT[:])
                    wcorner = ksb.tile([4, HP], BF16, tag="wcorner", name="wcorner")
                    # [c*16+t (part), l] -> dense DRAM -> [4, 2048] t-major -> DVE free-permute
                    wtmp = dram.tile([4 * HP], BF16, name="wtmp")
                    nc.scalar.dma_start(wtmp[:].rearrange("(p l) -> p l", p=64), wT4[:])
                    wcT = ksb.tile([4, HP], BF16, tag="wcT", name="wcT")
                    nc.scalar.dma_start(wcT[:], wtmp[:].rearrange("(c j) -> c j", c=4))
                    nc.vector.tensor_copy(wcorner[:].rearrange("c (l t) -> c l t", t=16),
                                          wcT[:].rearrange("c (t l) -> c l t", l=128))

                    # ---- P8: gathers (A|B and C|D stacked on partitions) ----
                    xab = ksb.tile([128, HP], F32, tag="xab", name="xab")
                    xcd = ksb.tile([128, HP], F32, tag="xcd", name="xcd")
                    nc.gpsimd.ap_gather(xab[:], kv2[:], idx1[:], channels=128,
                                        num_elems=4096, d=1, num_idxs=HP)
                    nc.gpsimd.ap_gather(xcd[:], kv2[:], idx2[:], channels=128,
                                        num_elems=4096, d=1, num_idxs=HP)
                    dump("d_xab", xab[:])

                    # ---- P9: corner-weight blend ----
                    for c0 in range(0, HP, 512):
                        sl = slice(c0, c0 + 512)
                        pw1 = kp.tile([128, 512], F32, tag="pa", name="pw1")
                        nc.tensor.matmul(pw1[:], ct["selAB"][:], wcorner[:, sl], start=True, stop=True)
                        pw2 = kp.tile([128, 512], F32, tag="pb", name="pw2")
                        nc.tensor.matmul(pw2[:], ct["selCD"][:], wcorner[:, sl], start=True, stop=True)
                        m1 = ksb.tile([128, 512], F32, tag="m1", name="m1")
                        nc.vector.tensor_tensor(m1[:], xab[:, sl], pw1[:], OP.mult)
                        m2 = ksb.tile([128, 512], F32, tag="m2", name="m2")
                        nc.vector.tensor_tensor(m2[:], xcd[:, sl], pw2[:], OP.mult)
                        nc.vector.tensor_tensor(x2sum[:, sl], m1[:], m2[:], OP.add)
                    dump("d_x2sum", x2sum[:])

                    # ---- P10: local k / vT projections ----
                    k_half = ksb.tile([64, HP], BF16, tag="xcd", name="k_half")
                    for c0 in range(0, HP, 512):
                        sl = slice(c0, c0 + 512)
                        pk = kp.tile([64, 512], F32, tag="pa", name="pk")
                        nc.tensor.matmul(pk[:], ct["wkT2"][:], x2sum[:, sl], start=True, stop=True)
                        nc.vector.tensor_copy(k_half[:, sl], pk[:])
                    vt_half = ksb.tile([128, 16 * 65], BF16, tag="xab", name="vt_half")
                    ones_col = vt_half[:].rearrange("p (t c) -> p t c", c=65)[:, :, 64]
                    nc.vector.memset(ones_col, 1.0)
                    for i in range(16):
                        pv = kp.tile([128, 64], F32, tag="pa", name="pv")
                        nc.tensor.matmul(pv[:], x2sum[:, 128 * i:128 * (i + 1)], ct["wvT2"][:],
                                         start=True, stop=True)
                        nc.vector.tensor_copy(vt_half[:, 65 * i:65 * i + 64], pv[:])

                    # ---- P11: exchange halves with pair core ----
                    KB = 64 * HP          # 131072 floats of k
                    VB = 128 * 16 * 65    # 133120 floats of vT
                    ex_in = dram.tile([KB + VB], BF16, name="ex_in")
                    ex_out = dram.tile([2, KB + VB], BF16, name="ex_out")
                    nc.sync.dma_start(ex_in[0:KB].rearrange("(p f) -> p f", p=64), k_half[:])
                    nc.sync.dma_start(ex_in[KB:KB + VB].rearrange("(p f) -> p f", p=128), vt_half[:])
                    nc.gpsimd.collective_compute(
                        "AllGather", OP.bypass,
                        replica_groups=[[0, 1], [2, 3], [4, 5], [6, 7]],
                        ins=[ex_in[:]], outs=[ex_out[:]],
                    )
                    for m in range(2):
                        ksrc = ex_out[m, 0:KB].rearrange("(p f) -> p f", p=64)
                        nc.sync.dma_start(kstack[0:64, 2048 * m:2048 * (m + 1)], ksrc)
                        vsrc = ex_out[m, KB:KB + VB].rearrange("(p f) -> p f", p=128)
                        nc.sync.dma_start(vt_all[:, 1040 * m:1040 * (m + 1)], vsrc)
                    dump("d_kstack", kstack[:])
                    dump("d_vt", vt_all[:])

            # ================= attention =================
            with nc.named_scope("attn"):
                with (
                    tc.tile_pool(name="aps", bufs=1, space="PSUM") as aps,
                    tc.tile_pool(name="apv", bufs=1, space="PSUM") as apv,
                    tc.tile_pool(name="apf", bufs=1, space="PSUM") as apf,
                    tc.tile_pool(name="asb", bufs=2) as asb,
                    tc.tile_pool(name="osb", bufs=1) as osb,
                ):
                    for mc in range(2):
                        qsl = slice(1024 * mc, 1024 * (mc + 1))
                        ps_av = apv.tile([65, 1024], F32, tag="ps_av", name="ps_av")
                        for pr_ in range(16):
                            ntA, ntB = 2 * pr_, 2 * pr_ + 1
                            sA = aps.tile([128, 1024], F32, tag="sA", name="sA")
                            sB = aps.tile([128, 1024], F32, tag="sB", name="sB")
                            for h in range(2):
                                hsl = slice(1024 * mc + 512 * h, 1024 * mc + 512 * (h + 1))
                                osl = slice(512 * h, 512 * (h + 1))
                                nc.tensor.matmul(sA[:, osl], kstack[0:64, 128 * ntA:128 * (ntA + 1)],
                                                 q2[0:64, hsl], start=True, stop=True)
                                nc.tensor.matmul(sB[:, osl], kstack[0:64, 128 * ntB:128 * (ntB + 1)],
                                                 q2[0:64, hsl], start=True, stop=True)
                            eA = asb.tile([128, 1024], BF16, tag="eA", name="eA")
                            nc.scalar.activation(eA[:], sA[:], AF.Exp, scale=0.125)
                            if mc == 0 and pr_ == 0 and debug:
                                s0c = osb.tile([128, 1024], F32, tag="s0c", name="s0c")
                                nc.vector.tensor_copy(s0c[:], sA[:])
                                dump("d_s0", s0c[:])
                                dump("d_e0", eA[:])
                            eB = asb.tile([128, 1024], BF16, tag="eB", name="eB")
                            nc.scalar.activation(eB[:], sB[:], AF.Exp, scale=0.125)
                            first = (pr_ == 0)
                            for h in range(2):
                                osl = slice(512 * h, 512 * (h + 1))
                                nc.tensor.matmul(ps_av[:, osl], vt_all[:, 65 * ntA:65 * (ntA + 1)],
                                                 eA[:, osl], start=first, stop=False,
                                                 skip_group_check=True)
                                nc.tensor.matmul(ps_av[:, osl], vt_all[:, 65 * ntB:65 * (ntB + 1)],
                                                 eB[:, osl], start=False, stop=(pr_ == 15),
                                                 skip_group_check=True)
                        # tail: normalize + output projection
                        rowc = osb.tile([1, 1024], F32, tag="rowc", name="rowc")
                        nc.vector.tensor_copy(rowc[:], ps_av[64:65, :])
                        r_rowf = osb.tile([1, 1024], F32, tag="r_rowf", name="r_rowf")
                        r_scr2 = osb.tile([1, 1024], F32, tag="r_scr2", name="r_scr2")
                        nc.vector.reciprocal_approx_accurate(r_rowf[:], rowc[:], r_scr2[:])
                        r_row = osb.tile([1, 1024], BF16, tag="r_row", name="r_row")
                        nc.vector.tensor_copy(r_row[:], r_rowf[:])
                        if mc == 0:
                            dump("d_rowsum", rowc[:])
                        ps_rb = apf.tile([64, 1024], F32, tag="ps_rb", name="ps_rb")
                        for h in range(2):
                            osl = slice(512 * h, 512 * (h + 1))
                            nc.tensor.matmul(ps_rb[:, osl], ct["b1"][:], r_row[:, osl],
                                             start=True, stop=True)
                        rbs = osb.tile([64, 1024], F32, tag="rbs", name="rbs")
                        nc.vector.tensor_copy(rbs[:], ps_rb[:])
                        onorm = osb.tile([64, 1024], BF16, tag="onorm", name="onorm")
                        nc.vector.tensor_tensor(onorm[:], ps_av[0:64, :], rbs[:], OP.mult)
                        ps_f = apf.tile([64, 1024], F32, tag="ps_rb", name="ps_f")
                        for h in range(2):
                            osl = slice(512 * h, 512 * (h + 1))
                            nc.tensor.matmul(ps_f[:, osl], ct["woT"][:], onorm[:, osl],
                                             start=True, stop=True)
                        osb_t = osb.tile([64, 1024], F32, tag="osb_t", name="osb_t")
                        nc.scalar.activation(osb_t[:], ps_f[:], AF.Identity, bias=ct["bo2_vec"][:])
                        nc.sync.dma_start(out_half.ap()[:, qsl], osb_t[:])

    nc.finalize()
    return nc, list(DBG.keys())


# ======================= host side =======================

BF16_KEYS = {"prompt_rows", "wqT", "dw_diag", "off_rhs", "ones_top", "ones_bot", "b1",
             "bneg", "selAB", "selCD", "wkT2", "wvT2", "woT"}


def prep_inputs(inputs):
    """inputs: full problem tensors (numpy). Returns list of 8 per-core dicts."""
    prompt = np.asarray(inputs["prompt"], np.float32)
    kv = np.asarray(inputs["kv"], np.float32)
    wq = np.asarray(inputs["wq"], np.float32); bq = np.asarray(inputs["bq"], np.float32)
    wk = np.asarray(inputs["wk"], np.float32)
    wv = np.asarray(inputs["wv"], np.float32); bv = np.asarray(inputs["bv"], np.float32)
    wo = np.asarray(inputs["wo"], np.float32); bo = np.asarray(inputs["bo"], np.float32)
    dw_w = np.asarray(inputs["dw_w"], np.float32); dw_b = np.asarray(inputs["dw_b"], np.float32)
    ln_w = np.asarray(inputs["ln_w"], np.float32); ln_b = np.asarray(inputs["ln_b"], np.float32)
    off_w = np.asarray(inputs["off_w"], np.float32)

    shared = {}
    shared["wqT"] = np.ascontiguousarray(wq.T)
    dwd = np.zeros((64, 9 * 64), np.float32)
    for tap in range(9):
        dy, dx = tap // 3, tap % 3
        dwd[:, 64 * tap:64 * (tap + 1)] = np.diag(dw_w[:, 0, dy, dx])
    shared["dw_diag"] = dwd
    shared["off_rhs"] = np.ascontiguousarray(off_w.T)  # [64,2]
    ot = np.zeros((128, 1), np.float32); ot[0:64] = 1.0
    ob_ = np.zeros((128, 1), np.float32); ob_[64:128] = 1.0
    shared["ones_top"] = ot; shared["ones_bot"] = ob_
    shared["b1"] = np.ones((1, 64), np.float32)
    shared["bneg"] = np.full((1, 64), -1.0 / 64.0, np.float32)
    sab = np.zeros((4, 128), np.float32); sab[0, 0:64] = 1.0; sab[1, 64:128] = 1.0
    scd = np.zeros((4, 128), np.float32); scd[2, 0:64] = 1.0; scd[3, 64:128] = 1.0
    shared["selAB"] = sab; shared["selCD"] = scd
    shared["wkT2"] = np.vstack([wk.T, wk.T]).astype(np.float32)
    shared["wvT2"] = np.vstack([wv.T, wv.T]).astype(np.float32)
    shared["woT"] = np.ascontiguousarray(wo.T)
    shared["ident"] = np.eye(128, dtype=np.float32)
    shared["dwb_vec"] = dw_b.reshape(64, 1)
    shared["lnw_vec"] = ln_w.reshape(64, 1)
    shared["lnb_vec"] = ln_b.reshape(64, 1)
    shared["bo2_vec"] = (wo @ bv + bo).reshape(64, 1)
    shared["eps_vec"] = np.full((1, 1), 1e-5, np.float32)

    maps = []
    for pid in range(8):
        b, hf = pid // 2, pid % 2
        r0 = 32 * hf
        pr = np.zeros((64, NROWS, 64), np.float32)
        bqm = np.zeros((64, NROWS, 64), np.float32)
        for ri in range(NROWS):
            r = r0 - 1 + ri
            if 0 <= r < 64:
                pr[:, ri] = prompt[b, :, r]
                bqm[:, ri] = bq[:, None]
        d = dict(shared)
        d["prompt_rows"] = pr.reshape(64, QCOLS)
        d["bq_map"] = bqm.reshape(64, QCOLS)
        d["kv2"] = np.vstack([kv[b].reshape(64, 4096)] * 2)
        rm = np.zeros((128, 32), np.float32)
        ll = np.arange(128)
        for t in range(16):
            p = t * 128 + ll
            rm[:, 2 * t] = r0 + p // 64 + 0.5
            rm[:, 2 * t + 1] = p % 64 + 0.5
        d["refmap"] = rm
        d = {k: (v.astype(ml_dtypes.bfloat16) if k in BF16_KEYS else v) for k, v in d.items()}
        maps.append(d)
    return maps


_CACHE = {}

def get_program(debug=False):
    key = bool(debug)
    if key not in _CACHE:
        _CACHE[key] = build_program(debug=debug)
    return _CACHE[key]


def kernel(**inputs):
    nc, _ = get_program(debug=False)
    maps = prep_inputs(inputs)
    res = run_bass_kernel_spmd(nc, maps, core_ids=list(range(8)))
    out = np.empty((B, 64, 64, 64), np.float32)
    for pid in range(8):
        b, hf = pid // 2, pid % 2
        out[b, :, 32 * hf:32 * hf + 32, :] = res.results[pid]["out_half"].reshape(64, 32, 64)
    return out

